# revision 20
# baseline (speedup 1.0000x reference)
"""Trainium2 Bass kernel for nn_Decoder (GRU decoder, B=64, T_FC=48, C=4096, HID=64).

Strategy
--------
Data-parallel over batch: 8 cores x 8 batch rows -> 32768 independent GRU
"columns" per core (batch*city on the free dim, features on partitions).

Host-side algebra folds fc_in and the autoregressive x_prev feedback into the
gate weights:
    G  = W_ih @ W_in                      [192, 4]
    gates_t = (W_hh + G[:,0:1] @ W_out) @ h_t + G[:,1:4] @ xt_t + const   (t>=1)
(with i_n / h_n kept separate for the r * h_n product).

Layout per 512-column chunk: hidden state lives in one [128, CHUNK] tile
(rows 0:64 = even-step h, 64:128 = odd-step h), which makes the output
projection a K=128 matmul covering two steps at once.  Biases enter through
activation bias vectors and a fused scalar_tensor_tensor.
"""

import os

import numpy as np

import concourse.bass as bass
import concourse.mybir as mybir
import concourse.tile as tile
from concourse import bacc
from concourse.bass_utils import run_bass_kernel_spmd

F32 = mybir.dt.float32
AF = mybir.ActivationFunctionType
ALU = mybir.AluOpType

B, T_HIST, T_FC, C, F_IN, HID = 64, 24, 48, 4096, 8, 64
N_CORES = 8
B_LOC = B // N_CORES
NCOLS = B_LOC * C  # 32768 columns per core
CHUNK = 512

_BUILT = {}
LAST_RESULTS = None  # BassKernelResults of the most recent run (for test.py)

W_SHAPES = {
    # h-side weights duplicated across both partition halves so odd steps
    # (h at rows 64:128) can use a matching lhsT base partition.
    "LRZ_H0": [128, 128], "LRZ_H1": [128, 128],
    "LN_H0": [128, 128], "LN_H1": [128, 128],
    "LRZ_X0": [4, 128], "LN_X0": [4, 128],
    "LRZ_X1": [3, 128], "LN_X1": [3, 128],
    "WOUT2": [128, 2],
    "BRZ0": [128, 1], "BRZ1": [128, 1],
    # per-partition bias vectors duplicated across both halves so either
    # parity's partition base reads the same values
    "BN0": [128, 1], "BN1": [128, 1],
    "BHHN": [128, 1], "BOUT2": [2, 1],
}


def _build(ncols, t_fc):
    key = (ncols, t_fc)
    if key in _BUILT:
        return _BUILT[key]

    nc = bacc.Bacc("TRN2", target_bir_lowering=False, debug=False,
                   num_devices=N_CORES)

    # XTD[k, t, col]: k<3 = decoder exogenous features for step t;
    # k=3 = xn at t=0 (zeros elsewhere).
    d_xtd = nc.dram_tensor("XTD", [4, t_fc, ncols], F32,
                           kind="ExternalInput").ap()
    d_ht = nc.dram_tensor("HT", [HID, ncols], F32, kind="ExternalInput").ap()
    d_w = {name: nc.dram_tensor(name, shape, F32, kind="ExternalInput").ap()
           for name, shape in W_SHAPES.items()}
    d_out = nc.dram_tensor("OUT", [t_fc, ncols], F32, kind="ExternalOutput").ap()

    nchunks = ncols // CHUNK

    with tile.TileContext(nc) as tc:
        with (
            tc.tile_pool(name="wpool", bufs=1) as wpool,
            tc.tile_pool(name="xpool", bufs=1) as xpool,
            tc.tile_pool(name="hpool", bufs=2) as hpool,
            tc.tile_pool(name="tpool", bufs=3) as tpool,
            tc.tile_pool(name="pspool", bufs=1, space="PSUM") as pspool,
        ):
            w = {}
            for name, ap in d_w.items():
                wt = wpool.tile(list(ap.shape), F32, name=f"w_{name}")
                nc.gpsimd.dma_start(wt[:], ap[:])
                w[name] = wt

            for ci in range(nchunks):
                cs = slice(ci * CHUNK, (ci + 1) * CHUNK)
                xtd = xpool.tile([4, t_fc, CHUNK], F32, tag="xtd")
                nc.gpsimd.dma_start(xtd[:], d_xtd[:, :, cs])

                hpair = hpool.tile([128, CHUNK], F32, tag="hpair")
                nc.gpsimd.dma_start(hpair[0:HID, :], d_ht[:, cs])

                for t in range(t_fc):
                    rb = (t % 2) * HID          # row base of h^(t)
                    wb = HID - rb               # row base of h^(t+1)
                    cur = hpair[rb:rb + HID, :]
                    if t == 0:
                        lrz_h, ln_h = w["LRZ_H0"], w["LN_H0"]
                        lrz_x, ln_x = w["LRZ_X0"], w["LN_X0"]
                        brz, bn = w["BRZ0"], w["BN0"]
                        xt_rhs = xtd[0:4, 0, :]
                    else:
                        lrz_h, ln_h = w["LRZ_H1"], w["LN_H1"]
                        lrz_x, ln_x = w["LRZ_X1"], w["LN_X1"]
                        brz, bn = w["BRZ1"], w["BN1"]
                        xt_rhs = xtd[0:3, t, :]

                    rzp = pspool.tile([128, CHUNK], F32, tag="rz", bufs=3)
                    npp = pspool.tile([128, CHUNK], F32, tag="n", bufs=3)
                    lrz_hs = lrz_h[rb:rb + HID, :]
                    ln_hs = ln_h[rb:rb + HID, :]
                    nc.tensor.matmul(rzp[:], lrz_hs, cur, start=True,
                                     stop=False)
                    nc.tensor.matmul(rzp[:], lrz_x[:], xt_rhs, start=False,
                                     stop=True)
                    nc.tensor.matmul(npp[:], ln_hs, cur, start=True,
                                     stop=False)
                    nc.tensor.matmul(npp[:], ln_x[:], xt_rhs, start=False,
                                     stop=True)

                    # [r; z] = sigmoid(rz psum + bias)
                    rzs = tpool.tile([128, CHUNK], F32, tag="rzs")
                    nc.scalar.activation(rzs[:], rzp[:], AF.Sigmoid,
                                         bias=brz[:])
                    # Temps are [128, CHUNK] sliced at the parity base rb so
                    # every both-SBUF tensor_tensor has matching base
                    # partitions (walrus birverifier constraint).
                    sl = slice(rb, rb + HID)
                    # r * (h_n + b_hh_n)
                    rhn = tpool.tile([128, CHUNK], F32, tag="rhn")
                    nc.vector.scalar_tensor_tensor(
                        rhn[sl, :], npp[HID:128, :], w["BHHN"][0:HID, :],
                        rzs[0:HID, :], op0=ALU.add, op1=ALU.mult)
                    # i_n + r*h_n
                    npre = tpool.tile([128, CHUNK], F32, tag="npre")
                    nc.vector.tensor_tensor(npre[sl, :], rhn[sl, :],
                                            npp[0:HID, :], op=ALU.add)
                    nt = tpool.tile([128, CHUNK], F32, tag="nt")
                    nc.scalar.activation(nt[sl, :], npre[sl, :], AF.Tanh,
                                         bias=bn[sl, :])
                    # h' = n + z*(h - n); hm parked at base 64 to meet z
                    hm = tpool.tile([128, CHUNK], F32, tag="hm")
                    nc.vector.tensor_tensor(hm[HID:128, :], cur, nt[sl, :],
                                            op=ALU.subtract)
                    zt = tpool.tile([128, CHUNK], F32, tag="zt")
                    nc.vector.tensor_tensor(zt[sl, :], rzs[HID:128, :],
                                            hm[HID:128, :], op=ALU.mult)
                    nc.vector.tensor_tensor(hpair[wb:wb + HID, :], nt[sl, :],
                                            zt[sl, :], op=ALU.add)

                    if t % 2 == 1:
                        # [pred_{t-1}; pred_t] = WOUT2.T @ [h^(t+1); h^(t)]
                        pp = pspool.tile([2, CHUNK], F32, tag="pred", bufs=2)
                        nc.tensor.matmul(pp[:], w["WOUT2"][:], hpair[:],
                                         start=True, stop=True)
                        pst = tpool.tile([2, CHUNK], F32, tag="pst")
                        nc.scalar.add(pst[:], pp[:], w["BOUT2"][:])
                        nc.gpsimd.dma_start(d_out[t - 1:t + 1, cs], pst[:])

    nc.compile()
    _BUILT[key] = nc
    return nc


def _prep_weights(W_in, b_in, W_ih, W_hh, b_ih, b_hh, W_out, b_out):
    f8 = np.float64
    G = W_ih.astype(f8) @ W_in.astype(f8)              # [192, 4]
    c = W_ih.astype(f8) @ b_in.astype(f8) + b_ih       # [192]
    wo = W_out.astype(f8)[0]                           # [64]
    bo = float(b_out[0])
    A1 = W_hh.astype(f8) + np.outer(G[:, 0], wo)       # [192, 64]
    d0 = c + b_hh                                      # [192]
    d1 = d0 + G[:, 0] * bo

    def dup(m):  # duplicate across both partition halves
        return np.concatenate([m, m], axis=0)

    w = {}
    w["LRZ_H1"] = dup(A1[:128].T)
    w["LRZ_H0"] = dup(W_hh[:128].astype(f8).T)

    ln_h1 = np.zeros((HID, 128), f8)
    ln_h1[:, 0:HID] = np.outer(wo, G[128:, 0])         # i_n feedback
    ln_h1[:, HID:128] = W_hh[128:].astype(f8).T        # h_n
    w["LN_H1"] = dup(ln_h1)
    ln_h0 = np.zeros((HID, 128), f8)
    ln_h0[:, HID:128] = W_hh[128:].astype(f8).T
    w["LN_H0"] = dup(ln_h0)

    w["LRZ_X1"] = G[:128, 1:4].T
    ln_x1 = np.zeros((3, 128), f8)
    ln_x1[:, 0:HID] = G[128:, 1:4].T
    w["LN_X1"] = ln_x1

    # step-0 rhs row order is [xt1, xt2, xt3, xn]
    perm = [1, 2, 3, 0]
    w["LRZ_X0"] = G[:128, perm].T
    ln_x0 = np.zeros((4, 128), f8)
    ln_x0[:, 0:HID] = G[128:, perm].T
    w["LN_X0"] = ln_x0

    wout2 = np.zeros((128, 2), f8)
    wout2[HID:128, 0] = wo                  # pred_{t-1} from odd rows h^(t)
    wout2[0:HID, 1] = wo                    # pred_t from even rows h^(t+1)
    w["WOUT2"] = wout2

    w["BRZ0"] = d0[:128, None]
    w["BRZ1"] = d1[:128, None]
    w["BN0"] = dup(c[128:, None])
    w["BN1"] = dup((c[128:] + G[128:, 0] * bo)[:, None])
    w["BHHN"] = dup(b_hh[128:].astype(f8)[:, None])
    w["BOUT2"] = np.full((2, 1), bo, f8)

    return {k: np.ascontiguousarray(v.astype(np.float32)) for k, v in w.items()}


def kernel(X, H, xn, W_in, b_in, W_ih, W_hh, b_ih, b_hh, W_out, b_out):
    global LAST_RESULTS
    X = np.asarray(X, np.float32)
    H = np.asarray(H, np.float32)
    xn = np.asarray(xn, np.float32)
    wmap = _prep_weights(np.asarray(W_in), np.asarray(b_in), np.asarray(W_ih),
                         np.asarray(W_hh), np.asarray(b_ih), np.asarray(b_hh),
                         np.asarray(W_out), np.asarray(b_out))

    Xs = X[:, T_HIST:T_HIST + T_FC, :, F_IN - 3:F_IN]  # [B, 48, C, 3]

    in_maps = []
    for ci in range(N_CORES):
        bs = slice(ci * B_LOC, (ci + 1) * B_LOC)
        Xc = Xs[bs]                                     # [8, 48, C, 3]
        XTD = np.zeros((4, T_FC, NCOLS), np.float32)
        XTD[0:3] = np.transpose(Xc, (3, 1, 0, 2)).reshape(3, T_FC, NCOLS)
        XTD[3, 0] = xn[bs, :, 0].reshape(NCOLS)
        HT = np.ascontiguousarray(
            H[bs].transpose(2, 0, 1).reshape(HID, NCOLS))
        m = {"XTD": XTD, "HT": HT}
        m.update(wmap)
        in_maps.append(m)

    nc = _build(NCOLS, T_FC)

    trace = os.environ.get("BASS_KERNEL_TRACE") == "1"
    if trace:
        _register_ntff_hook()
    res = run_bass_kernel_spmd(nc, in_maps, list(range(N_CORES)), trace=trace)
    LAST_RESULTS = res

    out = np.empty((B, T_FC, C, 1), np.float32)
    for ci in range(N_CORES):
        o = res.results[ci]["OUT"].reshape(T_FC, B_LOC, C)
        out[ci * B_LOC:(ci + 1) * B_LOC] = o.transpose(1, 0, 2)[..., None]
    return out


def _register_ntff_hook():
    """The agent image's antenv lacks axon_hooks; provide it so trace=True
    can capture NTFF profiles through libaxon_pjrt."""
    import sys
    import types
    if "antenv.axon_hooks" in sys.modules:
        return
    mod = types.ModuleType("antenv.axon_hooks")
    state = {"hook": None}
    mod.set_axon_ntff_profile_hook = lambda h: state.update(hook=h)
    mod.get_axon_ntff_profile_hook = lambda: state["hook"]
    sys.modules["antenv.axon_hooks"] = mod
    try:
        import antenv
        antenv.axon_hooks = mod
    except ImportError:
        pass
    try:
        from trn_agent_boot.trn_boot import _ntff_profile_via_ctypes
        hook = _ntff_profile_via_ctypes("/opt/axon/libaxon_pjrt.so")
        if hook is not None:
            mod.set_axon_ntff_profile_hook(hook)
    except Exception as e:  # pragma: no cover
        print(f"NTFF hook registration failed: {e}")
    # No artifact bucket in this sandbox; keep profiles local.
    import concourse.bass_utils as bu
    bu.upload_artifacts = lambda tmpdir: f"file://{tmpdir}"


# revision 22
# speedup vs baseline: 2.2112x; 2.2112x over previous
"""Trainium2 Bass kernel for nn_Decoder (GRU decoder, B=64, T_FC=48, C=4096, HID=64).

Strategy
--------
Data-parallel over batch: 8 cores x 8 batch rows -> 32768 independent GRU
"columns" per core (batch*city on the free dim, features on partitions).

Host-side algebra folds fc_in and the autoregressive x_prev feedback into the
gate weights:
    G  = W_ih @ W_in                      [192, 4]
    gates_t = (W_hh + G[:,0:1] @ W_out) @ h_t + G[:,1:4] @ xt_t + const   (t>=1)
(with i_n / h_n kept separate for the r * h_n product).

Layout per 512-column chunk: hidden state lives in one [128, CHUNK] tile
(rows 0:64 = even-step h, 64:128 = odd-step h), which makes the output
projection a K=128 matmul covering two steps at once.  Biases enter through
activation bias vectors and a fused scalar_tensor_tensor.
"""

import os

import numpy as np

import concourse.bass as bass
import concourse.mybir as mybir
import concourse.tile as tile
from concourse import bacc
from concourse.bass_utils import run_bass_kernel_spmd

F32 = mybir.dt.float32
BF16 = mybir.dt.bfloat16
AF = mybir.ActivationFunctionType
ALU = mybir.AluOpType

B, T_HIST, T_FC, C, F_IN, HID = 64, 24, 48, 4096, 8, 64
N_CORES = 8
B_LOC = B // N_CORES
NCOLS = B_LOC * C  # 32768 columns per core
CHUNK = 512

_BUILT = {}
LAST_RESULTS = None  # BassKernelResults of the most recent run (for test.py)

W_SHAPES = {
    # h-side weights duplicated across both partition halves so odd steps
    # (h at rows 64:128) can use a matching lhsT base partition.
    "LRZ_H0": [128, 128], "LRZ_H1": [128, 128],
    "LN_H0": [128, 128], "LN_H1": [128, 128],
    "LRZ_X0": [4, 128], "LN_X0": [4, 128],
    "LRZ_X1": [3, 128], "LN_X1": [3, 128],
    "WOUT2": [128, 2],
    "BRZ0": [128, 1], "BRZ1": [128, 1],
    # per-partition bias vectors duplicated across both halves so either
    # parity's partition base reads the same values
    "BN0": [128, 1], "BN1": [128, 1],
    "BHHN": [128, 1], "BOUT2": [2, 1],
}


def _build(ncols, t_fc):
    key = (ncols, t_fc)
    if key in _BUILT:
        return _BUILT[key]

    nc = bacc.Bacc("TRN2", target_bir_lowering=False, debug=False,
                   num_devices=N_CORES)

    # XTD[k, t, col]: k<3 = decoder exogenous features for step t;
    # k=3 = xn at t=0 (zeros elsewhere).
    d_xtd = nc.dram_tensor("XTD", [4, t_fc, ncols], BF16,
                           kind="ExternalInput").ap()
    d_ht = nc.dram_tensor("HT", [HID, ncols], BF16, kind="ExternalInput").ap()
    d_w = {name: nc.dram_tensor(name, shape,
                                F32 if name.startswith("B") else BF16,
                                kind="ExternalInput").ap()
           for name, shape in W_SHAPES.items()}
    d_out = nc.dram_tensor("OUT", [t_fc, ncols], F32, kind="ExternalOutput").ap()

    nchunks = ncols // CHUNK

    with tile.TileContext(nc) as tc:
        with (
            tc.tile_pool(name="wpool", bufs=1) as wpool,
            tc.tile_pool(name="xpool", bufs=1) as xpool,
            tc.tile_pool(name="hpool", bufs=2) as hpool,
            tc.tile_pool(name="tpool", bufs=3) as tpool,
            tc.tile_pool(name="pspool", bufs=1, space="PSUM") as pspool,
        ):
            w = {}
            for name, ap in d_w.items():
                wt = wpool.tile(list(ap.shape), ap.dtype, name=f"w_{name}")
                nc.gpsimd.dma_start(wt[:], ap[:])
                w[name] = wt

            for ci in range(nchunks):
                cs = slice(ci * CHUNK, (ci + 1) * CHUNK)
                xtd = xpool.tile([4, t_fc, CHUNK], BF16, tag="xtd", bufs=2)
                nc.gpsimd.dma_start(xtd[:], d_xtd[:, :, cs])

                hpair = hpool.tile([128, CHUNK], BF16, tag="hpair", bufs=4)
                nc.gpsimd.dma_start(hpair[0:HID, :], d_ht[:, cs])

                for t in range(t_fc):
                    rb = (t % 2) * HID          # row base of h^(t)
                    wb = HID - rb               # row base of h^(t+1)
                    cur = hpair[rb:rb + HID, :]
                    if t == 0:
                        lrz_h, ln_h = w["LRZ_H0"], w["LN_H0"]
                        lrz_x, ln_x = w["LRZ_X0"], w["LN_X0"]
                        brz, bn = w["BRZ0"], w["BN0"]
                        xt_rhs = xtd[0:4, 0, :]
                    else:
                        lrz_h, ln_h = w["LRZ_H1"], w["LN_H1"]
                        lrz_x, ln_x = w["LRZ_X1"], w["LN_X1"]
                        brz, bn = w["BRZ1"], w["BN1"]
                        xt_rhs = xtd[0:3, t, :]

                    rzp = pspool.tile([128, CHUNK], F32, tag="rz", bufs=3)
                    npp = pspool.tile([128, CHUNK], F32, tag="n", bufs=3)
                    lrz_hs = lrz_h[rb:rb + HID, :]
                    ln_hs = ln_h[rb:rb + HID, :]
                    nc.tensor.matmul(rzp[:], lrz_hs, cur, start=True,
                                     stop=False)
                    nc.tensor.matmul(rzp[:], lrz_x[:], xt_rhs, start=False,
                                     stop=True)
                    nc.tensor.matmul(npp[:], ln_hs, cur, start=True,
                                     stop=False)
                    nc.tensor.matmul(npp[:], ln_x[:], xt_rhs, start=False,
                                     stop=True)

                    # [r; z] = sigmoid(rz psum + bias)
                    rzs = tpool.tile([128, CHUNK], BF16, tag="rzs")
                    nc.scalar.activation(rzs[:], rzp[:], AF.Sigmoid,
                                         bias=brz[:])
                    # Temps are [128, CHUNK] sliced at the parity base rb so
                    # every both-SBUF tensor_tensor has matching base
                    # partitions (walrus birverifier constraint).
                    sl = slice(rb, rb + HID)
                    # r * (h_n + b_hh_n)
                    rhn = tpool.tile([128, CHUNK], BF16, tag="rhn")
                    nc.vector.scalar_tensor_tensor(
                        rhn[sl, :], npp[HID:128, :], w["BHHN"][0:HID, :],
                        rzs[0:HID, :], op0=ALU.add, op1=ALU.mult)
                    # i_n + r*h_n
                    npre = tpool.tile([128, CHUNK], BF16, tag="npre")
                    nc.vector.tensor_tensor(npre[sl, :], rhn[sl, :],
                                            npp[0:HID, :], op=ALU.add)
                    nt = tpool.tile([128, CHUNK], BF16, tag="nt")
                    nc.scalar.activation(nt[sl, :], npre[sl, :], AF.Tanh,
                                         bias=bn[sl, :])
                    # h' = n + z*(h - n); hm parked at base 64 to meet z
                    hm = tpool.tile([128, CHUNK], BF16, tag="hm")
                    nc.vector.tensor_tensor(hm[HID:128, :], cur, nt[sl, :],
                                            op=ALU.subtract)
                    zt = tpool.tile([128, CHUNK], BF16, tag="zt")
                    nc.vector.tensor_tensor(zt[sl, :], rzs[HID:128, :],
                                            hm[HID:128, :], op=ALU.mult)
                    nc.vector.tensor_tensor(hpair[wb:wb + HID, :], nt[sl, :],
                                            zt[sl, :], op=ALU.add)

                    if t % 2 == 1:
                        # [pred_{t-1}; pred_t] = WOUT2.T @ [h^(t+1); h^(t)]
                        pp = pspool.tile([2, CHUNK], F32, tag="pred", bufs=2)
                        nc.tensor.matmul(pp[:], w["WOUT2"][:], hpair[:],
                                         start=True, stop=True)
                        pst = tpool.tile([2, CHUNK], F32, tag="pst")
                        nc.scalar.add(pst[:], pp[:], w["BOUT2"][:])
                        nc.gpsimd.dma_start(d_out[t - 1:t + 1, cs], pst[:])

    nc.compile()
    _BUILT[key] = nc
    return nc


def _prep_weights(W_in, b_in, W_ih, W_hh, b_ih, b_hh, W_out, b_out):
    f8 = np.float64
    G = W_ih.astype(f8) @ W_in.astype(f8)              # [192, 4]
    c = W_ih.astype(f8) @ b_in.astype(f8) + b_ih       # [192]
    wo = W_out.astype(f8)[0]                           # [64]
    bo = float(b_out[0])
    A1 = W_hh.astype(f8) + np.outer(G[:, 0], wo)       # [192, 64]
    d0 = c + b_hh                                      # [192]
    d1 = d0 + G[:, 0] * bo

    def dup(m):  # duplicate across both partition halves
        return np.concatenate([m, m], axis=0)

    w = {}
    w["LRZ_H1"] = dup(A1[:128].T)
    w["LRZ_H0"] = dup(W_hh[:128].astype(f8).T)

    ln_h1 = np.zeros((HID, 128), f8)
    ln_h1[:, 0:HID] = np.outer(wo, G[128:, 0])         # i_n feedback
    ln_h1[:, HID:128] = W_hh[128:].astype(f8).T        # h_n
    w["LN_H1"] = dup(ln_h1)
    ln_h0 = np.zeros((HID, 128), f8)
    ln_h0[:, HID:128] = W_hh[128:].astype(f8).T
    w["LN_H0"] = dup(ln_h0)

    w["LRZ_X1"] = G[:128, 1:4].T
    ln_x1 = np.zeros((3, 128), f8)
    ln_x1[:, 0:HID] = G[128:, 1:4].T
    w["LN_X1"] = ln_x1

    # step-0 rhs row order is [xt1, xt2, xt3, xn]
    perm = [1, 2, 3, 0]
    w["LRZ_X0"] = G[:128, perm].T
    ln_x0 = np.zeros((4, 128), f8)
    ln_x0[:, 0:HID] = G[128:, perm].T
    w["LN_X0"] = ln_x0

    wout2 = np.zeros((128, 2), f8)
    wout2[HID:128, 0] = wo                  # pred_{t-1} from odd rows h^(t)
    wout2[0:HID, 1] = wo                    # pred_t from even rows h^(t+1)
    w["WOUT2"] = wout2

    w["BRZ0"] = d0[:128, None]
    w["BRZ1"] = d1[:128, None]
    w["BN0"] = dup(c[128:, None])
    w["BN1"] = dup((c[128:] + G[128:, 0] * bo)[:, None])
    w["BHHN"] = dup(b_hh[128:].astype(f8)[:, None])
    w["BOUT2"] = np.full((2, 1), bo, f8)

    import ml_dtypes
    return {k: np.ascontiguousarray(
        v.astype(np.float32 if k.startswith("B") else ml_dtypes.bfloat16))
        for k, v in w.items()}


def kernel(X, H, xn, W_in, b_in, W_ih, W_hh, b_ih, b_hh, W_out, b_out):
    global LAST_RESULTS
    X = np.asarray(X, np.float32)
    H = np.asarray(H, np.float32)
    xn = np.asarray(xn, np.float32)
    wmap = _prep_weights(np.asarray(W_in), np.asarray(b_in), np.asarray(W_ih),
                         np.asarray(W_hh), np.asarray(b_ih), np.asarray(b_hh),
                         np.asarray(W_out), np.asarray(b_out))

    Xs = X[:, T_HIST:T_HIST + T_FC, :, F_IN - 3:F_IN]  # [B, 48, C, 3]

    in_maps = []
    for ci in range(N_CORES):
        bs = slice(ci * B_LOC, (ci + 1) * B_LOC)
        Xc = Xs[bs]                                     # [8, 48, C, 3]
        import ml_dtypes
        XTD = np.zeros((4, T_FC, NCOLS), ml_dtypes.bfloat16)
        XTD[0:3] = np.transpose(Xc, (3, 1, 0, 2)).reshape(3, T_FC, NCOLS)
        XTD[3, 0] = xn[bs, :, 0].reshape(NCOLS)
        HT = np.ascontiguousarray(
            H[bs].transpose(2, 0, 1).reshape(HID, NCOLS).astype(ml_dtypes.bfloat16))
        m = {"XTD": XTD, "HT": HT}
        m.update(wmap)
        in_maps.append(m)

    nc = _build(NCOLS, T_FC)

    trace = os.environ.get("BASS_KERNEL_TRACE") == "1"
    if trace:
        _register_ntff_hook()
    res = run_bass_kernel_spmd(nc, in_maps, list(range(N_CORES)), trace=trace)
    LAST_RESULTS = res

    out = np.empty((B, T_FC, C, 1), np.float32)
    for ci in range(N_CORES):
        o = res.results[ci]["OUT"].reshape(T_FC, B_LOC, C)
        out[ci * B_LOC:(ci + 1) * B_LOC] = o.transpose(1, 0, 2)[..., None]
    return out


def _register_ntff_hook():
    """The agent image's antenv lacks axon_hooks; provide it so trace=True
    can capture NTFF profiles through libaxon_pjrt."""
    import sys
    import types
    if "antenv.axon_hooks" in sys.modules:
        return
    mod = types.ModuleType("antenv.axon_hooks")
    state = {"hook": None}
    mod.set_axon_ntff_profile_hook = lambda h: state.update(hook=h)
    mod.get_axon_ntff_profile_hook = lambda: state["hook"]
    sys.modules["antenv.axon_hooks"] = mod
    try:
        import antenv
        antenv.axon_hooks = mod
    except ImportError:
        pass
    try:
        from trn_agent_boot.trn_boot import _ntff_profile_via_ctypes
        hook = _ntff_profile_via_ctypes("/opt/axon/libaxon_pjrt.so")
        if hook is not None:
            mod.set_axon_ntff_profile_hook(hook)
    except Exception as e:  # pragma: no cover
        print(f"NTFF hook registration failed: {e}")
    # No artifact bucket in this sandbox; keep profiles local.
    import concourse.bass_utils as bu
    bu.upload_artifacts = lambda tmpdir: f"file://{tmpdir}"


# revision 24
# speedup vs baseline: 4.8177x; 2.1788x over previous
"""Trainium2 Bass kernel for nn_Decoder (GRU decoder, B=64, T_FC=48, C=4096, HID=64).

Strategy
--------
Data-parallel over batch: 8 cores x 8 batch rows -> 32768 independent GRU
"columns" per core (batch*city on the free dim, features on partitions).

Host-side algebra folds fc_in and the autoregressive x_prev feedback into the
gate weights:
    G  = W_ih @ W_in                      [192, 4]
    gates_t = (W_hh + G[:,0:1] @ W_out) @ h_t + G[:,1:4] @ xt_t + const   (t>=1)
(with i_n / h_n kept separate for the r * h_n product).

Layout per 512-column chunk: hidden state lives in one [128, CHUNK] tile
(rows 0:64 = even-step h, 64:128 = odd-step h), which makes the output
projection a K=128 matmul covering two steps at once.  Biases enter through
activation bias vectors and a fused scalar_tensor_tensor.
"""

import os

import numpy as np

import concourse.bass as bass
import concourse.mybir as mybir
import concourse.tile as tile
from concourse import bacc
from concourse.bass_utils import run_bass_kernel_spmd

F32 = mybir.dt.float32
BF16 = mybir.dt.bfloat16
AF = mybir.ActivationFunctionType
ALU = mybir.AluOpType

B, T_HIST, T_FC, C, F_IN, HID = 64, 24, 48, 4096, 8, 64
N_CORES = 8
B_LOC = B // N_CORES
NCOLS = B_LOC * C  # 32768 columns per core
CHUNK = 512

_BUILT = {}
LAST_RESULTS = None  # BassKernelResults of the most recent run (for test.py)

W_SHAPES = {
    # h-side weights duplicated across both partition halves so odd steps
    # (h at rows 64:128) can use a matching lhsT base partition.
    "LRZ_H0": [128, 128], "LRZ_H1": [128, 128],
    "LN_H0": [128, 128], "LN_H1": [128, 128],
    "LRZ_X0": [4, 128], "LN_X0": [4, 128],
    "LRZ_X1": [3, 128], "LN_X1": [3, 128],
    "WOUT2": [128, 2],
    "BRZ0": [128, 1], "BRZ1": [128, 1],
    # per-partition bias vectors duplicated across both halves so either
    # parity's partition base reads the same values
    "BN0": [128, 1], "BN1": [128, 1],
    "BHHN": [128, 1], "BOUT2": [2, 1],
}


def _build(ncols, t_fc):
    key = (ncols, t_fc)
    if key in _BUILT:
        return _BUILT[key]

    nc = bacc.Bacc("TRN2", target_bir_lowering=False, debug=False,
                   num_devices=N_CORES)

    # XTD[k, t, col]: k<3 = decoder exogenous features for step t;
    # k=3 = xn at t=0 (zeros elsewhere).
    d_xtd = nc.dram_tensor("XTD", [4, t_fc, ncols], BF16,
                           kind="ExternalInput").ap()
    d_ht = nc.dram_tensor("HT", [HID, ncols], BF16, kind="ExternalInput").ap()
    d_w = {name: nc.dram_tensor(name, shape,
                                F32 if name.startswith("B") else BF16,
                                kind="ExternalInput").ap()
           for name, shape in W_SHAPES.items()}
    d_out = nc.dram_tensor("OUT", [t_fc, ncols], F32, kind="ExternalOutput").ap()

    nchunks = ncols // CHUNK

    with tile.TileContext(nc) as tc:
        with (
            tc.tile_pool(name="wpool", bufs=1) as wpool,
            tc.tile_pool(name="xpool", bufs=1) as xpool,
            tc.tile_pool(name="hpool", bufs=2) as hpool,
            tc.tile_pool(name="tpool", bufs=6) as tpool,
            tc.tile_pool(name="pspool", bufs=1, space="PSUM") as pspool,
        ):
            w = {}
            for name, ap in d_w.items():
                wt = wpool.tile(list(ap.shape), ap.dtype, name=f"w_{name}")
                nc.gpsimd.dma_start(wt[:], ap[:])
                w[name] = wt

            IL = 4       # chunks processed in lockstep
            XB = 8       # xt steps per DMA block
            for g in range(0, nchunks, IL):
                group = list(range(g, min(g + IL, nchunks)))
                st = {}
                for ci in group:
                    cs = slice(ci * CHUNK, (ci + 1) * CHUNK)
                    xt0 = xpool.tile([4, CHUNK], BF16, tag="xt0",
                                     bufs=IL + 2)
                    nc.gpsimd.dma_start(xt0[:], d_xtd[:, 0, cs])
                    hpair = hpool.tile([128, CHUNK], BF16, tag="hpair",
                                       bufs=IL + 2)
                    nc.gpsimd.dma_start(hpair[0:HID, :], d_ht[:, cs])
                    st[ci] = {"cs": cs, "xt0": xt0, "hpair": hpair,
                              "xtb": None}

                for t in range(t_fc):
                  for ci in group:
                    cs = st[ci]["cs"]
                    hpair = st[ci]["hpair"]
                    if t % XB == 0 and t + 1 < t_fc:
                        # exogenous features for steps t..t+XB-1
                        xtb = xpool.tile([3, XB, CHUNK], BF16, tag="xtb",
                                         bufs=3 * IL)
                        nc.gpsimd.dma_start(
                            xtb[:], d_xtd[0:3, t:t + XB, cs])
                        st[ci]["xtb"] = xtb
                    rb = (t % 2) * HID          # row base of h^(t)
                    wb = HID - rb               # row base of h^(t+1)
                    cur = hpair[rb:rb + HID, :]
                    if t == 0:
                        lrz_h, ln_h = w["LRZ_H0"], w["LN_H0"]
                        lrz_x, ln_x = w["LRZ_X0"], w["LN_X0"]
                        brz, bn = w["BRZ0"], w["BN0"]
                        xt_rhs = st[ci]["xt0"][0:4, :]
                    else:
                        lrz_h, ln_h = w["LRZ_H1"], w["LN_H1"]
                        lrz_x, ln_x = w["LRZ_X1"], w["LN_X1"]
                        brz, bn = w["BRZ1"], w["BN1"]
                        xt_rhs = st[ci]["xtb"][0:3, t % XB, :]

                    rzp = pspool.tile([128, CHUNK], F32, tag="rz", bufs=3)
                    npp = pspool.tile([128, CHUNK], F32, tag="n", bufs=3)
                    lrz_hs = lrz_h[rb:rb + HID, :]
                    ln_hs = ln_h[rb:rb + HID, :]
                    nc.tensor.matmul(rzp[:], lrz_hs, cur, start=True,
                                     stop=False)
                    nc.tensor.matmul(rzp[:], lrz_x[:], xt_rhs, start=False,
                                     stop=True)
                    nc.tensor.matmul(npp[:], ln_hs, cur, start=True,
                                     stop=False)
                    nc.tensor.matmul(npp[:], ln_x[:], xt_rhs, start=False,
                                     stop=True)

                    # [r; z] = sigmoid(rz psum + bias)
                    rzs = tpool.tile([128, CHUNK], BF16, tag="rzs")
                    nc.scalar.activation(rzs[:], rzp[:], AF.Sigmoid,
                                         bias=brz[:])
                    # Temps are [128, CHUNK] sliced at the parity base rb so
                    # every both-SBUF tensor_tensor has matching base
                    # partitions (walrus birverifier constraint).
                    sl = slice(rb, rb + HID)
                    # r * (h_n + b_hh_n)
                    rhn = tpool.tile([128, CHUNK], BF16, tag="rhn")
                    nc.vector.scalar_tensor_tensor(
                        rhn[sl, :], npp[HID:128, :], w["BHHN"][0:HID, :],
                        rzs[0:HID, :], op0=ALU.add, op1=ALU.mult)
                    # i_n + r*h_n
                    npre = tpool.tile([128, CHUNK], BF16, tag="npre")
                    nc.vector.tensor_tensor(npre[sl, :], rhn[sl, :],
                                            npp[0:HID, :], op=ALU.add)
                    nt = tpool.tile([128, CHUNK], BF16, tag="nt")
                    nc.scalar.activation(nt[sl, :], npre[sl, :], AF.Tanh,
                                         bias=bn[sl, :])
                    # h' = n + z*(h - n); hm parked at base 64 to meet z
                    hm = tpool.tile([128, CHUNK], BF16, tag="hm")
                    nc.vector.tensor_tensor(hm[HID:128, :], cur, nt[sl, :],
                                            op=ALU.subtract)
                    zt = tpool.tile([128, CHUNK], BF16, tag="zt")
                    nc.vector.tensor_tensor(zt[sl, :], rzs[HID:128, :],
                                            hm[HID:128, :], op=ALU.mult)
                    nc.vector.tensor_tensor(hpair[wb:wb + HID, :], nt[sl, :],
                                            zt[sl, :], op=ALU.add)

                    if t % 2 == 1:
                        # [pred_{t-1}; pred_t] = WOUT2.T @ [h^(t+1); h^(t)]
                        pp = pspool.tile([2, CHUNK], F32, tag="pred", bufs=2)
                        nc.tensor.matmul(pp[:], w["WOUT2"][:], hpair[:],
                                         start=True, stop=True)
                        pst = tpool.tile([2, CHUNK], F32, tag="pst")
                        nc.scalar.add(pst[:], pp[:], w["BOUT2"][:])
                        nc.gpsimd.dma_start(d_out[t - 1:t + 1, cs], pst[:])

    nc.compile()
    _BUILT[key] = nc
    return nc


def _prep_weights(W_in, b_in, W_ih, W_hh, b_ih, b_hh, W_out, b_out):
    f8 = np.float64
    G = W_ih.astype(f8) @ W_in.astype(f8)              # [192, 4]
    c = W_ih.astype(f8) @ b_in.astype(f8) + b_ih       # [192]
    wo = W_out.astype(f8)[0]                           # [64]
    bo = float(b_out[0])
    A1 = W_hh.astype(f8) + np.outer(G[:, 0], wo)       # [192, 64]
    d0 = c + b_hh                                      # [192]
    d1 = d0 + G[:, 0] * bo

    def dup(m):  # duplicate across both partition halves
        return np.concatenate([m, m], axis=0)

    w = {}
    w["LRZ_H1"] = dup(A1[:128].T)
    w["LRZ_H0"] = dup(W_hh[:128].astype(f8).T)

    ln_h1 = np.zeros((HID, 128), f8)
    ln_h1[:, 0:HID] = np.outer(wo, G[128:, 0])         # i_n feedback
    ln_h1[:, HID:128] = W_hh[128:].astype(f8).T        # h_n
    w["LN_H1"] = dup(ln_h1)
    ln_h0 = np.zeros((HID, 128), f8)
    ln_h0[:, HID:128] = W_hh[128:].astype(f8).T
    w["LN_H0"] = dup(ln_h0)

    w["LRZ_X1"] = G[:128, 1:4].T
    ln_x1 = np.zeros((3, 128), f8)
    ln_x1[:, 0:HID] = G[128:, 1:4].T
    w["LN_X1"] = ln_x1

    # step-0 rhs row order is [xt1, xt2, xt3, xn]
    perm = [1, 2, 3, 0]
    w["LRZ_X0"] = G[:128, perm].T
    ln_x0 = np.zeros((4, 128), f8)
    ln_x0[:, 0:HID] = G[128:, perm].T
    w["LN_X0"] = ln_x0

    wout2 = np.zeros((128, 2), f8)
    wout2[HID:128, 0] = wo                  # pred_{t-1} from odd rows h^(t)
    wout2[0:HID, 1] = wo                    # pred_t from even rows h^(t+1)
    w["WOUT2"] = wout2

    w["BRZ0"] = d0[:128, None]
    w["BRZ1"] = d1[:128, None]
    w["BN0"] = dup(c[128:, None])
    w["BN1"] = dup((c[128:] + G[128:, 0] * bo)[:, None])
    w["BHHN"] = dup(b_hh[128:].astype(f8)[:, None])
    w["BOUT2"] = np.full((2, 1), bo, f8)

    import ml_dtypes
    return {k: np.ascontiguousarray(
        v.astype(np.float32 if k.startswith("B") else ml_dtypes.bfloat16))
        for k, v in w.items()}


def kernel(X, H, xn, W_in, b_in, W_ih, W_hh, b_ih, b_hh, W_out, b_out):
    global LAST_RESULTS
    X = np.asarray(X, np.float32)
    H = np.asarray(H, np.float32)
    xn = np.asarray(xn, np.float32)
    wmap = _prep_weights(np.asarray(W_in), np.asarray(b_in), np.asarray(W_ih),
                         np.asarray(W_hh), np.asarray(b_ih), np.asarray(b_hh),
                         np.asarray(W_out), np.asarray(b_out))

    Xs = X[:, T_HIST:T_HIST + T_FC, :, F_IN - 3:F_IN]  # [B, 48, C, 3]

    in_maps = []
    for ci in range(N_CORES):
        bs = slice(ci * B_LOC, (ci + 1) * B_LOC)
        Xc = Xs[bs]                                     # [8, 48, C, 3]
        import ml_dtypes
        XTD = np.zeros((4, T_FC, NCOLS), ml_dtypes.bfloat16)
        XTD[0:3] = np.transpose(Xc, (3, 1, 0, 2)).reshape(3, T_FC, NCOLS)
        XTD[3, 0] = xn[bs, :, 0].reshape(NCOLS)
        HT = np.ascontiguousarray(
            H[bs].transpose(2, 0, 1).reshape(HID, NCOLS).astype(ml_dtypes.bfloat16))
        m = {"XTD": XTD, "HT": HT}
        m.update(wmap)
        in_maps.append(m)

    nc = _build(NCOLS, T_FC)

    trace = os.environ.get("BASS_KERNEL_TRACE") == "1"
    if trace:
        _register_ntff_hook()
    res = run_bass_kernel_spmd(nc, in_maps, list(range(N_CORES)), trace=trace)
    LAST_RESULTS = res

    out = np.empty((B, T_FC, C, 1), np.float32)
    for ci in range(N_CORES):
        o = res.results[ci]["OUT"].reshape(T_FC, B_LOC, C)
        out[ci * B_LOC:(ci + 1) * B_LOC] = o.transpose(1, 0, 2)[..., None]
    return out


def _register_ntff_hook():
    """The agent image's antenv lacks axon_hooks; provide it so trace=True
    can capture NTFF profiles through libaxon_pjrt."""
    import sys
    import types
    if "antenv.axon_hooks" in sys.modules:
        return
    mod = types.ModuleType("antenv.axon_hooks")
    state = {"hook": None}
    mod.set_axon_ntff_profile_hook = lambda h: state.update(hook=h)
    mod.get_axon_ntff_profile_hook = lambda: state["hook"]
    sys.modules["antenv.axon_hooks"] = mod
    try:
        import antenv
        antenv.axon_hooks = mod
    except ImportError:
        pass
    try:
        from trn_agent_boot.trn_boot import _ntff_profile_via_ctypes
        hook = _ntff_profile_via_ctypes("/opt/axon/libaxon_pjrt.so")
        if hook is not None:
            mod.set_axon_ntff_profile_hook(hook)
    except Exception as e:  # pragma: no cover
        print(f"NTFF hook registration failed: {e}")
    # No artifact bucket in this sandbox; keep profiles local.
    import concourse.bass_utils as bu
    bu.upload_artifacts = lambda tmpdir: f"file://{tmpdir}"


# revision 27
# speedup vs baseline: 5.0671x; 1.0518x over previous
"""Trainium2 Bass kernel for nn_Decoder (GRU decoder, B=64, T_FC=48, C=4096, HID=64).

Strategy
--------
Data-parallel over batch: 8 cores x 8 batch rows -> 32768 independent GRU
"columns" per core (batch*city on the free dim, features on partitions).

Host-side algebra folds fc_in and the autoregressive x_prev feedback into the
gate weights:
    G  = W_ih @ W_in                      [192, 4]
    gates_t = (W_hh + G[:,0:1] @ W_out) @ h_t + G[:,1:4] @ xt_t + const   (t>=1)
(with i_n / h_n kept separate for the r * h_n product).

Layout per 512-column chunk: hidden state lives in one [128, CHUNK] tile
(rows 0:64 = even-step h, 64:128 = odd-step h), which makes the output
projection a K=128 matmul covering two steps at once.  Biases enter through
activation bias vectors and a fused scalar_tensor_tensor.
"""

import os

import numpy as np

import concourse.bass as bass
import concourse.mybir as mybir
import concourse.tile as tile
from concourse import bacc
from concourse.bass_utils import run_bass_kernel_spmd

F32 = mybir.dt.float32
BF16 = mybir.dt.bfloat16
AF = mybir.ActivationFunctionType
ALU = mybir.AluOpType

B, T_HIST, T_FC, C, F_IN, HID = 64, 24, 48, 4096, 8, 64
N_CORES = 8
B_LOC = B // N_CORES
NCOLS = B_LOC * C  # 32768 columns per core
CHUNK = 512

_BUILT = {}
LAST_RESULTS = None  # BassKernelResults of the most recent run (for test.py)

W_SHAPES = {
    # h-side weights duplicated across both partition halves so odd steps
    # (h at rows 64:128) can use a matching lhsT base partition.
    "LRZ_H0": [128, 128], "LRZ_H1": [128, 128],
    "LN_H0": [128, 128], "LN_H1": [128, 128],
    "LRZ_X0": [4, 128], "LN_X0": [4, 128],
    "LRZ_X1": [3, 128], "LN_X1": [3, 128],
    "WOUT2": [128, 2],
    "BRZ0": [128, 1], "BRZ1": [128, 1],
    # per-partition bias vectors duplicated across both halves so either
    # parity's partition base reads the same values
    "BN0": [128, 1], "BN1": [128, 1],
    "BHHN": [128, 1], "BOUT2": [2, 1],
}


def _build(ncols, t_fc):
    key = (ncols, t_fc)
    if key in _BUILT:
        return _BUILT[key]

    nc = bacc.Bacc("TRN2", target_bir_lowering=False, debug=False,
                   num_devices=N_CORES)

    # XTD[k, t, col]: k<3 = decoder exogenous features for step t;
    # k=3 = xn at t=0 (zeros elsewhere).
    d_xtd = nc.dram_tensor("XTD", [4, t_fc, ncols], BF16,
                           kind="ExternalInput").ap()
    d_ht = nc.dram_tensor("HT", [HID, ncols], BF16, kind="ExternalInput").ap()
    d_w = {name: nc.dram_tensor(name, shape,
                                F32 if name.startswith("B") else BF16,
                                kind="ExternalInput").ap()
           for name, shape in W_SHAPES.items()}
    d_out = nc.dram_tensor("OUT", [t_fc, ncols], F32, kind="ExternalOutput").ap()

    nchunks = ncols // CHUNK

    with tile.TileContext(nc) as tc:
        with (
            tc.tile_pool(name="wpool", bufs=1) as wpool,
            tc.tile_pool(name="xpool", bufs=1) as xpool,
            tc.tile_pool(name="hpool", bufs=2) as hpool,
            tc.tile_pool(name="tpool", bufs=6) as tpool,
            tc.tile_pool(name="pspool", bufs=1, space="PSUM") as pspool,
        ):
            w = {}
            for name, ap in d_w.items():
                wt = wpool.tile(list(ap.shape), ap.dtype, name=f"w_{name}")
                nc.gpsimd.dma_start(wt[:], ap[:])
                w[name] = wt

            IL = 4       # chunks processed in lockstep
            XB = 8       # xt steps per DMA block
            for g in range(0, nchunks, IL):
                group = list(range(g, min(g + IL, nchunks)))
                st = {}
                for ci in group:
                    cs = slice(ci * CHUNK, (ci + 1) * CHUNK)
                    xt0 = xpool.tile([4, CHUNK], BF16, tag="xt0",
                                     bufs=IL + 2)
                    nc.gpsimd.dma_start(xt0[:], d_xtd[:, 0, cs])
                    hpair = hpool.tile([128, CHUNK], BF16, tag="hpair",
                                       bufs=IL + 2)
                    nc.gpsimd.dma_start(hpair[0:HID, :], d_ht[:, cs])
                    st[ci] = {"cs": cs, "xt0": xt0, "hpair": hpair,
                              "xtb": None}

                for t in range(t_fc):
                  for ci in group:
                    cs = st[ci]["cs"]
                    hpair = st[ci]["hpair"]
                    if t % XB == 0 and t + 1 < t_fc:
                        # exogenous features for steps t..t+XB-1
                        xtb = xpool.tile([3, XB, CHUNK], BF16, tag="xtb",
                                         bufs=3 * IL)
                        nc.gpsimd.dma_start(
                            xtb[:], d_xtd[0:3, t:t + XB, cs])
                        st[ci]["xtb"] = xtb
                    rb = (t % 2) * HID          # row base of h^(t)
                    wb = HID - rb               # row base of h^(t+1)
                    cur = hpair[rb:rb + HID, :]
                    if t == 0:
                        lrz_h, ln_h = w["LRZ_H0"], w["LN_H0"]
                        lrz_x, ln_x = w["LRZ_X0"], w["LN_X0"]
                        brz, bn = w["BRZ0"], w["BN0"]
                        xt_rhs = st[ci]["xt0"][0:4, :]
                    else:
                        lrz_h, ln_h = w["LRZ_H1"], w["LN_H1"]
                        lrz_x, ln_x = w["LRZ_X1"], w["LN_X1"]
                        brz, bn = w["BRZ1"], w["BN1"]
                        xt_rhs = st[ci]["xtb"][0:3, t % XB, :]

                    rzp = pspool.tile([128, CHUNK], F32, tag="rz", bufs=3)
                    npp = pspool.tile([128, CHUNK], F32, tag="n", bufs=3)
                    lrz_hs = lrz_h[rb:rb + HID, :]
                    ln_hs = ln_h[rb:rb + HID, :]
                    nc.tensor.matmul(rzp[:], lrz_hs, cur, start=True,
                                     stop=False)
                    nc.tensor.matmul(rzp[:], lrz_x[:], xt_rhs, start=False,
                                     stop=True)
                    nc.tensor.matmul(npp[:], ln_hs, cur, start=True,
                                     stop=False)
                    nc.tensor.matmul(npp[:], ln_x[:], xt_rhs, start=False,
                                     stop=True)

                    # [z; r] = sigmoid(rz psum + bias)  (z rows 0:64)
                    rzs = tpool.tile([128, CHUNK], BF16, tag="rzs")
                    nc.scalar.activation(rzs[:], rzp[:], AF.Sigmoid,
                                         bias=brz[:])
                    # Evacuate [i_n; h_n] psum in one ACT op; bias vector
                    # adds b_hh_n to the h_n half only.
                    nsb = tpool.tile([128, CHUNK], BF16, tag="nsb")
                    nc.scalar.activation(nsb[:], npp[:], AF.Identity,
                                         bias=w["BHHN"][:])
                    sl = slice(rb, rb + HID)
                    # r * (h_n + b_hh_n)   (both operands at base 64)
                    rhn = tpool.tile([128, CHUNK], BF16, tag="rhn")
                    nc.vector.tensor_tensor(rhn[0:HID, :], nsb[HID:128, :],
                                            rzs[HID:128, :], op=ALU.mult)
                    # i_n + r*h_n          (both at base 0)
                    npre = tpool.tile([128, CHUNK], BF16, tag="npre")
                    nc.vector.tensor_tensor(npre[0:HID, :], rhn[0:HID, :],
                                            nsb[0:HID, :], op=ALU.add)
                    nt = tpool.tile([128, CHUNK], BF16, tag="nt")
                    nc.scalar.activation(nt[sl, :], npre[0:HID, :], AF.Tanh,
                                         bias=bn[0:HID, :])
                    # h' = n + z*(h - n)
                    hm = tpool.tile([128, CHUNK], BF16, tag="hm")
                    nc.vector.tensor_tensor(hm[0:HID, :], cur, nt[sl, :],
                                            op=ALU.subtract)
                    zt = tpool.tile([128, CHUNK], BF16, tag="zt")
                    nc.vector.tensor_tensor(zt[sl, :], rzs[0:HID, :],
                                            hm[0:HID, :], op=ALU.mult)
                    nc.vector.tensor_tensor(hpair[wb:wb + HID, :], nt[sl, :],
                                            zt[sl, :], op=ALU.add)

                    if t % 2 == 1:
                        # [pred_{t-1}; pred_t] = WOUT2.T @ [h^(t+1); h^(t)]
                        pp = pspool.tile([2, CHUNK], F32, tag="pred", bufs=2)
                        nc.tensor.matmul(pp[:], w["WOUT2"][:], hpair[:],
                                         start=True, stop=True)
                        pst = tpool.tile([2, CHUNK], F32, tag="pst")
                        nc.scalar.add(pst[:], pp[:], w["BOUT2"][:])
                        nc.gpsimd.dma_start(d_out[t - 1:t + 1, cs], pst[:])

    nc.compile()
    _BUILT[key] = nc
    return nc


def _prep_weights(W_in, b_in, W_ih, W_hh, b_ih, b_hh, W_out, b_out):
    f8 = np.float64
    G = W_ih.astype(f8) @ W_in.astype(f8)              # [192, 4]
    c = W_ih.astype(f8) @ b_in.astype(f8) + b_ih       # [192]
    wo = W_out.astype(f8)[0]                           # [64]
    bo = float(b_out[0])
    A1 = W_hh.astype(f8) + np.outer(G[:, 0], wo)       # [192, 64]
    d0 = c + b_hh                                      # [192]
    d1 = d0 + G[:, 0] * bo

    def dup(m):  # duplicate across both partition halves
        return np.concatenate([m, m], axis=0)

    def rzswap(m):  # [*,128] gate cols: [r;z] -> [z;r]
        return np.concatenate([m[:, HID:128], m[:, 0:HID]], axis=1)

    w = {}
    w["LRZ_H1"] = dup(rzswap(A1[:128].T))
    w["LRZ_H0"] = dup(rzswap(W_hh[:128].astype(f8).T))

    ln_h1 = np.zeros((HID, 128), f8)
    ln_h1[:, 0:HID] = np.outer(wo, G[128:, 0])         # i_n feedback
    ln_h1[:, HID:128] = W_hh[128:].astype(f8).T        # h_n
    w["LN_H1"] = dup(ln_h1)
    ln_h0 = np.zeros((HID, 128), f8)
    ln_h0[:, HID:128] = W_hh[128:].astype(f8).T
    w["LN_H0"] = dup(ln_h0)

    w["LRZ_X1"] = rzswap(G[:128, 1:4].T)
    ln_x1 = np.zeros((3, 128), f8)
    ln_x1[:, 0:HID] = G[128:, 1:4].T
    w["LN_X1"] = ln_x1

    # step-0 rhs row order is [xt1, xt2, xt3, xn]
    perm = [1, 2, 3, 0]
    w["LRZ_X0"] = rzswap(G[:128, perm].T)
    ln_x0 = np.zeros((4, 128), f8)
    ln_x0[:, 0:HID] = G[128:, perm].T
    w["LN_X0"] = ln_x0

    wout2 = np.zeros((128, 2), f8)
    wout2[HID:128, 0] = wo                  # pred_{t-1} from odd rows h^(t)
    wout2[0:HID, 1] = wo                    # pred_t from even rows h^(t+1)
    w["WOUT2"] = wout2

    def rzswap_v(v):
        return np.concatenate([v[HID:128], v[0:HID]], axis=0)

    w["BRZ0"] = rzswap_v(d0[:128])[:, None]
    w["BRZ1"] = rzswap_v(d1[:128])[:, None]
    w["BN0"] = dup(c[128:, None])
    w["BN1"] = dup((c[128:] + G[128:, 0] * bo)[:, None])
    bhhn = np.zeros((128, 1), f8)
    bhhn[HID:128, 0] = b_hh[128:]
    w["BHHN"] = bhhn
    w["BOUT2"] = np.full((2, 1), bo, f8)

    import ml_dtypes
    return {k: np.ascontiguousarray(
        v.astype(np.float32 if k.startswith("B") else ml_dtypes.bfloat16))
        for k, v in w.items()}


def kernel(X, H, xn, W_in, b_in, W_ih, W_hh, b_ih, b_hh, W_out, b_out):
    global LAST_RESULTS
    X = np.asarray(X, np.float32)
    H = np.asarray(H, np.float32)
    xn = np.asarray(xn, np.float32)
    wmap = _prep_weights(np.asarray(W_in), np.asarray(b_in), np.asarray(W_ih),
                         np.asarray(W_hh), np.asarray(b_ih), np.asarray(b_hh),
                         np.asarray(W_out), np.asarray(b_out))

    Xs = X[:, T_HIST:T_HIST + T_FC, :, F_IN - 3:F_IN]  # [B, 48, C, 3]

    in_maps = []
    for ci in range(N_CORES):
        bs = slice(ci * B_LOC, (ci + 1) * B_LOC)
        Xc = Xs[bs]                                     # [8, 48, C, 3]
        import ml_dtypes
        XTD = np.zeros((4, T_FC, NCOLS), ml_dtypes.bfloat16)
        XTD[0:3] = np.transpose(Xc, (3, 1, 0, 2)).reshape(3, T_FC, NCOLS)
        XTD[3, 0] = xn[bs, :, 0].reshape(NCOLS)
        HT = np.ascontiguousarray(
            H[bs].transpose(2, 0, 1).reshape(HID, NCOLS).astype(ml_dtypes.bfloat16))
        m = {"XTD": XTD, "HT": HT}
        m.update(wmap)
        in_maps.append(m)

    nc = _build(NCOLS, T_FC)

    trace = os.environ.get("BASS_KERNEL_TRACE") == "1"
    if trace:
        _register_ntff_hook()
    res = run_bass_kernel_spmd(nc, in_maps, list(range(N_CORES)), trace=trace)
    LAST_RESULTS = res

    out = np.empty((B, T_FC, C, 1), np.float32)
    for ci in range(N_CORES):
        o = res.results[ci]["OUT"].reshape(T_FC, B_LOC, C)
        out[ci * B_LOC:(ci + 1) * B_LOC] = o.transpose(1, 0, 2)[..., None]
    return out


def _register_ntff_hook():
    """The agent image's antenv lacks axon_hooks; provide it so trace=True
    can capture NTFF profiles through libaxon_pjrt."""
    import sys
    import types
    if "antenv.axon_hooks" in sys.modules:
        return
    mod = types.ModuleType("antenv.axon_hooks")
    state = {"hook": None}
    mod.set_axon_ntff_profile_hook = lambda h: state.update(hook=h)
    mod.get_axon_ntff_profile_hook = lambda: state["hook"]
    sys.modules["antenv.axon_hooks"] = mod
    try:
        import antenv
        antenv.axon_hooks = mod
    except ImportError:
        pass
    try:
        from trn_agent_boot.trn_boot import _ntff_profile_via_ctypes
        hook = _ntff_profile_via_ctypes("/opt/axon/libaxon_pjrt.so")
        if hook is not None:
            mod.set_axon_ntff_profile_hook(hook)
    except Exception as e:  # pragma: no cover
        print(f"NTFF hook registration failed: {e}")
    # No artifact bucket in this sandbox; keep profiles local.
    import concourse.bass_utils as bu
    bu.upload_artifacts = lambda tmpdir: f"file://{tmpdir}"


# revision 30
# speedup vs baseline: 5.1822x; 1.0227x over previous
"""Trainium2 Bass kernel for nn_Decoder (GRU decoder, B=64, T_FC=48, C=4096, HID=64).

Strategy
--------
Data-parallel over batch: 8 cores x 8 batch rows -> 32768 independent GRU
"columns" per core (batch*city on the free dim, features on partitions).

Host-side algebra folds fc_in and the autoregressive x_prev feedback into the
gate weights:
    G  = W_ih @ W_in                      [192, 4]
    gates_t = (W_hh + G[:,0:1] @ W_out) @ h_t + G[:,1:4] @ xt_t + const   (t>=1)
(with i_n / h_n kept separate for the r * h_n product).

Layout per 512-column chunk: hidden state lives in one [128, CHUNK] tile
(rows 0:64 = even-step h, 64:128 = odd-step h), which makes the output
projection a K=128 matmul covering two steps at once.  Biases enter through
activation bias vectors and a fused scalar_tensor_tensor.
"""

import os

import numpy as np

import concourse.bass as bass
import concourse.mybir as mybir
import concourse.tile as tile
from concourse import bacc
from concourse.bass_utils import run_bass_kernel_spmd

F32 = mybir.dt.float32
BF16 = mybir.dt.bfloat16
AF = mybir.ActivationFunctionType
ALU = mybir.AluOpType

B, T_HIST, T_FC, C, F_IN, HID = 64, 24, 48, 4096, 8, 64
N_CORES = 8
B_LOC = B // N_CORES
NCOLS = B_LOC * C  # 32768 columns per core
CHUNK = 512

_BUILT = {}
LAST_RESULTS = None  # BassKernelResults of the most recent run (for test.py)

W_SHAPES = {
    # h-side weights duplicated across both partition halves so odd steps
    # (h at rows 64:128) can use a matching lhsT base partition.
    "LRZ_H0": [128, 128], "LRZ_H1": [128, 128],
    "LN_H0": [128, 128], "LN_H1": [128, 128],
    "LRZ_X0": [4, 128], "LN_X0": [4, 128],
    "LRZ_X1": [3, 128], "LN_X1": [3, 128],
    "WOUT2": [128, 2],
    "BRZ0": [128, 1], "BRZ1": [128, 1],
    # per-partition bias vectors duplicated across both halves so either
    # parity's partition base reads the same values
    "BN0": [128, 1], "BN1": [128, 1],
    "BHHN": [128, 1], "BOUT2": [2, 1],
}


def _build(ncols, t_fc):
    key = (ncols, t_fc)
    if key in _BUILT:
        return _BUILT[key]

    nc = bacc.Bacc("TRN2", target_bir_lowering=False, debug=False,
                   num_devices=N_CORES)

    # XTD[k, t, col]: k<3 = decoder exogenous features for step t;
    # k=3 = xn at t=0 (zeros elsewhere).
    d_xtd = nc.dram_tensor("XTD", [4, t_fc, ncols], BF16,
                           kind="ExternalInput").ap()
    d_ht = nc.dram_tensor("HT", [HID, ncols], BF16, kind="ExternalInput").ap()
    d_w = {name: nc.dram_tensor(name, shape,
                                F32 if name.startswith("B") else BF16,
                                kind="ExternalInput").ap()
           for name, shape in W_SHAPES.items()}
    d_out = nc.dram_tensor("OUT", [t_fc, ncols], F32, kind="ExternalOutput").ap()

    nchunks = ncols // CHUNK

    with tile.TileContext(nc) as tc:
        with (
            tc.tile_pool(name="wpool", bufs=1) as wpool,
            tc.tile_pool(name="xpool", bufs=1) as xpool,
            tc.tile_pool(name="hpool", bufs=2) as hpool,
            tc.tile_pool(name="tpool", bufs=8) as tpool,
            tc.tile_pool(name="pspool", bufs=1, space="PSUM") as pspool,
        ):
            w = {}
            for name, ap in d_w.items():
                wt = wpool.tile(list(ap.shape), ap.dtype, name=f"w_{name}")
                nc.gpsimd.dma_start(wt[:], ap[:])
                w[name] = wt

            IL = 8       # chunks processed in lockstep
            XB = 4       # xt steps per DMA block
            for g in range(0, nchunks, IL):
                group = list(range(g, min(g + IL, nchunks)))
                st = {}
                for ci in group:
                    cs = slice(ci * CHUNK, (ci + 1) * CHUNK)
                    xt0 = xpool.tile([4, CHUNK], BF16, tag="xt0",
                                     bufs=IL + 2)
                    nc.gpsimd.dma_start(xt0[:], d_xtd[:, 0, cs])
                    hpair = hpool.tile([128, CHUNK], BF16, tag="hpair",
                                       bufs=IL + 2)
                    nc.gpsimd.dma_start(hpair[0:HID, :], d_ht[:, cs])
                    st[ci] = {"cs": cs, "xt0": xt0, "hpair": hpair,
                              "xtb": None}

                for t in range(t_fc):
                  for ci in group:
                    cs = st[ci]["cs"]
                    hpair = st[ci]["hpair"]
                    if t % XB == 0 and t + 1 < t_fc:
                        # exogenous features for steps t..t+XB-1
                        xtb = xpool.tile([3, XB, CHUNK], BF16, tag="xtb",
                                         bufs=3 * IL)
                        nc.gpsimd.dma_start(
                            xtb[:], d_xtd[0:3, t:t + XB, cs])
                        st[ci]["xtb"] = xtb
                    rb = (t % 2) * HID          # row base of h^(t)
                    wb = HID - rb               # row base of h^(t+1)
                    cur = hpair[rb:rb + HID, :]
                    if t == 0:
                        lrz_h, ln_h = w["LRZ_H0"], w["LN_H0"]
                        lrz_x, ln_x = w["LRZ_X0"], w["LN_X0"]
                        brz, bn = w["BRZ0"], w["BN0"]
                        xt_rhs = st[ci]["xt0"][0:4, :]
                    else:
                        lrz_h, ln_h = w["LRZ_H1"], w["LN_H1"]
                        lrz_x, ln_x = w["LRZ_X1"], w["LN_X1"]
                        brz, bn = w["BRZ1"], w["BN1"]
                        xt_rhs = st[ci]["xtb"][0:3, t % XB, :]

                    rzp = pspool.tile([128, CHUNK], F32, tag="rz", bufs=3)
                    npp = pspool.tile([128, CHUNK], F32, tag="n", bufs=3)
                    lrz_hs = lrz_h[rb:rb + HID, :]
                    ln_hs = ln_h[rb:rb + HID, :]
                    nc.tensor.matmul(rzp[:], lrz_hs, cur, start=True,
                                     stop=False)
                    nc.tensor.matmul(rzp[:], lrz_x[:], xt_rhs, start=False,
                                     stop=True)
                    nc.tensor.matmul(npp[:], ln_hs, cur, start=True,
                                     stop=False)
                    nc.tensor.matmul(npp[:], ln_x[:], xt_rhs, start=False,
                                     stop=True)

                    # [z; r] = sigmoid(rz psum + bias)  (z rows 0:64)
                    rzs = tpool.tile([128, CHUNK], BF16, tag="rzs")
                    nc.scalar.activation(rzs[:], rzp[:], AF.Sigmoid,
                                         bias=brz[:])
                    # Evacuate [i_n; h_n] psum in one ACT op; bias vector
                    # adds b_hh_n to the h_n half only.
                    nsb = tpool.tile([128, CHUNK], BF16, tag="nsb")
                    nc.scalar.activation(nsb[:], npp[:], AF.Identity,
                                         bias=w["BHHN"][:])
                    sl = slice(rb, rb + HID)
                    # r * (h_n + b_hh_n)   (both operands at base 64)
                    rhn = tpool.tile([128, CHUNK], BF16, tag="rhn")
                    nc.vector.tensor_tensor(rhn[0:HID, :], nsb[HID:128, :],
                                            rzs[HID:128, :], op=ALU.mult)
                    # i_n + r*h_n          (both at base 0)
                    npre = tpool.tile([128, CHUNK], BF16, tag="npre")
                    nc.vector.tensor_tensor(npre[0:HID, :], rhn[0:HID, :],
                                            nsb[0:HID, :], op=ALU.add)
                    nt = tpool.tile([128, CHUNK], BF16, tag="nt")
                    nc.scalar.activation(nt[sl, :], npre[0:HID, :], AF.Tanh,
                                         bias=bn[0:HID, :])
                    # h' = n + z*(h - n)
                    hm = tpool.tile([128, CHUNK], BF16, tag="hm")
                    nc.vector.tensor_tensor(hm[0:HID, :], cur, nt[sl, :],
                                            op=ALU.subtract)
                    zt = tpool.tile([128, CHUNK], BF16, tag="zt")
                    nc.vector.tensor_tensor(zt[sl, :], rzs[0:HID, :],
                                            hm[0:HID, :], op=ALU.mult)
                    nc.vector.tensor_tensor(hpair[wb:wb + HID, :], nt[sl, :],
                                            zt[sl, :], op=ALU.add)

                    if t % 2 == 1:
                        # [pred_{t-1}; pred_t] = WOUT2.T @ [h^(t+1); h^(t)]
                        pp = pspool.tile([2, CHUNK], F32, tag="pred", bufs=2)
                        nc.tensor.matmul(pp[:], w["WOUT2"][:], hpair[:],
                                         start=True, stop=True)
                        pst = tpool.tile([2, CHUNK], F32, tag="pst")
                        nc.scalar.add(pst[:], pp[:], w["BOUT2"][:])
                        nc.gpsimd.dma_start(d_out[t - 1:t + 1, cs], pst[:])

    nc.compile()
    _BUILT[key] = nc
    return nc


def _prep_weights(W_in, b_in, W_ih, W_hh, b_ih, b_hh, W_out, b_out):
    f8 = np.float64
    G = W_ih.astype(f8) @ W_in.astype(f8)              # [192, 4]
    c = W_ih.astype(f8) @ b_in.astype(f8) + b_ih       # [192]
    wo = W_out.astype(f8)[0]                           # [64]
    bo = float(b_out[0])
    A1 = W_hh.astype(f8) + np.outer(G[:, 0], wo)       # [192, 64]
    d0 = c + b_hh                                      # [192]
    d1 = d0 + G[:, 0] * bo

    def dup(m):  # duplicate across both partition halves
        return np.concatenate([m, m], axis=0)

    def rzswap(m):  # [*,128] gate cols: [r;z] -> [z;r]
        return np.concatenate([m[:, HID:128], m[:, 0:HID]], axis=1)

    w = {}
    w["LRZ_H1"] = dup(rzswap(A1[:128].T))
    w["LRZ_H0"] = dup(rzswap(W_hh[:128].astype(f8).T))

    ln_h1 = np.zeros((HID, 128), f8)
    ln_h1[:, 0:HID] = np.outer(wo, G[128:, 0])         # i_n feedback
    ln_h1[:, HID:128] = W_hh[128:].astype(f8).T        # h_n
    w["LN_H1"] = dup(ln_h1)
    ln_h0 = np.zeros((HID, 128), f8)
    ln_h0[:, HID:128] = W_hh[128:].astype(f8).T
    w["LN_H0"] = dup(ln_h0)

    w["LRZ_X1"] = rzswap(G[:128, 1:4].T)
    ln_x1 = np.zeros((3, 128), f8)
    ln_x1[:, 0:HID] = G[128:, 1:4].T
    w["LN_X1"] = ln_x1

    # step-0 rhs row order is [xt1, xt2, xt3, xn]
    perm = [1, 2, 3, 0]
    w["LRZ_X0"] = rzswap(G[:128, perm].T)
    ln_x0 = np.zeros((4, 128), f8)
    ln_x0[:, 0:HID] = G[128:, perm].T
    w["LN_X0"] = ln_x0

    wout2 = np.zeros((128, 2), f8)
    wout2[HID:128, 0] = wo                  # pred_{t-1} from odd rows h^(t)
    wout2[0:HID, 1] = wo                    # pred_t from even rows h^(t+1)
    w["WOUT2"] = wout2

    def rzswap_v(v):
        return np.concatenate([v[HID:128], v[0:HID]], axis=0)

    w["BRZ0"] = rzswap_v(d0[:128])[:, None]
    w["BRZ1"] = rzswap_v(d1[:128])[:, None]
    w["BN0"] = dup(c[128:, None])
    w["BN1"] = dup((c[128:] + G[128:, 0] * bo)[:, None])
    bhhn = np.zeros((128, 1), f8)
    bhhn[HID:128, 0] = b_hh[128:]
    w["BHHN"] = bhhn
    w["BOUT2"] = np.full((2, 1), bo, f8)

    import ml_dtypes
    return {k: np.ascontiguousarray(
        v.astype(np.float32 if k.startswith("B") else ml_dtypes.bfloat16))
        for k, v in w.items()}


def kernel(X, H, xn, W_in, b_in, W_ih, W_hh, b_ih, b_hh, W_out, b_out):
    global LAST_RESULTS
    X = np.asarray(X, np.float32)
    H = np.asarray(H, np.float32)
    xn = np.asarray(xn, np.float32)
    wmap = _prep_weights(np.asarray(W_in), np.asarray(b_in), np.asarray(W_ih),
                         np.asarray(W_hh), np.asarray(b_ih), np.asarray(b_hh),
                         np.asarray(W_out), np.asarray(b_out))

    Xs = X[:, T_HIST:T_HIST + T_FC, :, F_IN - 3:F_IN]  # [B, 48, C, 3]

    in_maps = []
    for ci in range(N_CORES):
        bs = slice(ci * B_LOC, (ci + 1) * B_LOC)
        Xc = Xs[bs]                                     # [8, 48, C, 3]
        import ml_dtypes
        XTD = np.zeros((4, T_FC, NCOLS), ml_dtypes.bfloat16)
        XTD[0:3] = np.transpose(Xc, (3, 1, 0, 2)).reshape(3, T_FC, NCOLS)
        XTD[3, 0] = xn[bs, :, 0].reshape(NCOLS)
        HT = np.ascontiguousarray(
            H[bs].transpose(2, 0, 1).reshape(HID, NCOLS).astype(ml_dtypes.bfloat16))
        m = {"XTD": XTD, "HT": HT}
        m.update(wmap)
        in_maps.append(m)

    nc = _build(NCOLS, T_FC)

    trace = os.environ.get("BASS_KERNEL_TRACE") == "1"
    if trace:
        _register_ntff_hook()
    res = run_bass_kernel_spmd(nc, in_maps, list(range(N_CORES)), trace=trace)
    LAST_RESULTS = res

    out = np.empty((B, T_FC, C, 1), np.float32)
    for ci in range(N_CORES):
        o = res.results[ci]["OUT"].reshape(T_FC, B_LOC, C)
        out[ci * B_LOC:(ci + 1) * B_LOC] = o.transpose(1, 0, 2)[..., None]
    return out


def _register_ntff_hook():
    """The agent image's antenv lacks axon_hooks; provide it so trace=True
    can capture NTFF profiles through libaxon_pjrt."""
    import sys
    import types
    if "antenv.axon_hooks" in sys.modules:
        return
    mod = types.ModuleType("antenv.axon_hooks")
    state = {"hook": None}
    mod.set_axon_ntff_profile_hook = lambda h: state.update(hook=h)
    mod.get_axon_ntff_profile_hook = lambda: state["hook"]
    sys.modules["antenv.axon_hooks"] = mod
    try:
        import antenv
        antenv.axon_hooks = mod
    except ImportError:
        pass
    try:
        from trn_agent_boot.trn_boot import _ntff_profile_via_ctypes
        hook = _ntff_profile_via_ctypes("/opt/axon/libaxon_pjrt.so")
        if hook is not None:
            mod.set_axon_ntff_profile_hook(hook)
    except Exception as e:  # pragma: no cover
        print(f"NTFF hook registration failed: {e}")
    # No artifact bucket in this sandbox; keep profiles local.
    import concourse.bass_utils as bu
    bu.upload_artifacts = lambda tmpdir: f"file://{tmpdir}"
